# revision 1
# baseline (speedup 1.0000x reference)
"""Trainium2 Bass kernel for nn_CLUB_816043786555 (CLUB loss).

Full-input contract: kernel(**inputs) takes the complete arrays, shards the
batch dim across 8 NeuronCores, runs a Bass/Tile kernel per core, and
combines tiny per-core partial sums on the host.

Math: with mu = leaky(x@W1m+b1m)@W2m+b2m, logvar = tanh(leaky(x@W1v+b1v)@W2v+b2v),
iv = exp(-logvar), ym_d = mean_i y, y2m_d = mean_i y^2:

  loss = -0.5/N * sum_{i,d} iv*(y^2 - 2*mu*y - y2m + 2*mu*ym)
       = -0.5/N * [ P1 - 2*P2 - sum_d y2m_d*B_d + 2*sum_d ym_d*C_d ]

with per-core partials P1 = sum iv*y^2, P2 = sum iv*mu*y, C_d = sum_i iv*mu,
B_d = sum_i iv, S_d = sum_i y, T_d = sum_i y^2.  All partials are produced
on-device as fp32 accumulations; the host combine is O(128) work.

Precision: x, y, W, and hidden activations are fp16 (PE runs fp16 matmuls at
full rate; fp32 PSUM accumulation); mu, iv, and the product stage stay fp32.
Measured 6.4e-3 relative error on the final scalar vs the f32 reference
(the loss is a ~1e5x cancelling sum, so precision placement is load-bearing).
"""

import numpy as np

N_CORES = 8
N = 131072
D = 128
X_DIM = 128
H2 = 512
M = N // N_CORES          # rows per core = 16384
RG = 1024                 # rows per group
NG = M // RG              # groups per core = 16
NEG_SLOPE = 0.2

# Leaky-evacuation split knob: of every 10 (group, mlp, chunk) units,
# DVE_UNITS of them run on DVE (single fused custom op) instead of ACT.
DVE_UNITS = (0, 2, 4, 6)


def _dve_leaky(g, k, c):
    return ((g * 2 + k) * 4 + c) % 10 in DVE_UNITS


# engine for the P1/P2 product ops: gpsimd frees DVE for leaky evacuation
PROD_ON_POOL = False


def _peng(nc):
    return nc.gpsimd if PROD_ON_POOL else nc.vector


_leaky_op = None


def _get_leaky_op():
    """Custom DVE uop: out = max((in0 + s0) * imm2, in0 + s0) — fused
    bias-add + leaky-relu in one 1x pass straight from PSUM."""
    global _leaky_op
    if _leaky_op is not None:
        return _leaky_op
    import concourse.dve_ops as DO
    from concourse.dve_spec import C0, C2, Spec, Src0, maxx

    op = DO.DveOp(
        "LEAKY_BIAS_ANT",
        Spec(
            body=maxx((Src0 + C0) * C2, Src0 + C0),
            reference=lambda in0, in1, s0, s1, imm2: np.maximum(
                (in0.astype(np.float32) + s0) * imm2,
                in0.astype(np.float32) + s0),
        ),
        subdim=False,
        uops_sha={"v3": "28ce115f5da0f06f", "v4": ""},
    )
    DO.OPS.append(op)
    DO.CUSTOM_DVE_SPECS[op.name] = op.spec
    DO._SUB_OPCODE_FOR_NAME[op.name] = DO._CUSTOM_DVE_ROW_BASE + len(DO.OPS) - 1
    assert DO._SUB_OPCODE_FOR_NAME[op.name] < 0x20
    _leaky_op = op
    return op

_compiled = None


def _build():
    import concourse.bacc as bacc
    import concourse.tile as tile
    import concourse.mybir as mybir

    F32 = mybir.dt.float32
    F16 = mybir.dt.float16
    AF = mybir.ActivationFunctionType
    OP = mybir.AluOpType

    nc = bacc.Bacc("TRN2", target_bir_lowering=False, debug=False,
                   num_devices=N_CORES)

    x_d = nc.dram_tensor("x", [M, X_DIM], F32, kind="ExternalInput")
    y_d = nc.dram_tensor("y", [M, D], F32, kind="ExternalInput")
    w1_d = [nc.dram_tensor("W1m", [X_DIM, H2], F32, kind="ExternalInput"),
            nc.dram_tensor("W1v", [X_DIM, H2], F32, kind="ExternalInput")]
    b1_d = [nc.dram_tensor("b1m", [H2], F32, kind="ExternalInput"),
            nc.dram_tensor("b1v", [H2], F32, kind="ExternalInput")]
    w2_d = [nc.dram_tensor("W2m", [H2, D], F32, kind="ExternalInput"),
            nc.dram_tensor("W2v", [H2, D], F32, kind="ExternalInput")]
    b2_d = [nc.dram_tensor("b2m", [D], F32, kind="ExternalInput"),
            nc.dram_tensor("b2v", [D], F32, kind="ExternalInput")]
    out_d = nc.dram_tensor("out", [6, D, NG], F32, kind="ExternalOutput")

    with tile.TileContext(nc) as tc:
        with (
            tc.tile_pool(name="singles", bufs=1) as singles,
            tc.tile_pool(name="tposed", bufs=2) as tposed,
            tc.tile_pool(name="hidden", bufs=2) as hidden,
            tc.tile_pool(name="l2", bufs=2) as l2pool,
            tc.tile_pool(name="scratch", bufs=2) as scratch,
            tc.tile_pool(name="hpsum", bufs=2, space="PSUM") as hpsum,
            tc.tile_pool(name="l2psum", bufs=1, space="PSUM") as l2psum,
            tc.tile_pool(name="dram", bufs=1, space="DRAM") as dram,
        ):
            # fp16 row-major bounce buffers in DRAM (see loop below).
            xh_dram = dram.tile([M, X_DIM], F16, name="xh_dram")
            yh_dram = dram.tile([M, D], F16, name="yh_dram")

            def cast_and_transpose(g, xT, yT, nsub):
                # gpsimd (SWDGE) DMAs can cast, so one DRAM->DRAM casting
                # DMA per group produces the fp16 rows, and one big
                # DRAM->SBUF xbar transpose per group loads them as
                # [feature, row].  Per-instruction DGE overhead (~0.6-1us)
                # makes small transposes far more expensive than big ones.
                sub = RG // nsub
                for i in range(nsub):
                    rows = slice(g * RG + i * sub, g * RG + (i + 1) * sub)
                    nc.gpsimd.dma_start(xh_dram[rows, :], x_d[rows, :])
                    nc.gpsimd.dma_start(yh_dram[rows, :], y_d[rows, :])
                    nc.sync.dma_start_transpose(
                        xT[:, i * sub:(i + 1) * sub], xh_dram[rows, :])
                    nc.sync.dma_start_transpose(
                        yT[:, i * sub:(i + 1) * sub], yh_dram[rows, :])

            # Group 0's cast+transpose chain is emitted FIRST (quartered) so
            # the DMA engines deliver the first xT slab before the pile of
            # weight/bias loads — the first matmul only needs w1 and xT[q0].
            xT0 = tposed.tile([X_DIM, RG], F16, tag="xT", name="xT0")
            yT0 = tposed.tile([D, RG], F16, tag="yT", name="yT0")
            cast_and_transpose(0, xT0, yT0, nsub=4)

            # --- weights / biases: load f32, cast weights to fp16 ---
            w1h, w2h, b1t, b2t, nb2t = [], [], [], [], []
            for k in range(2):
                w1f = singles.tile([X_DIM, H2], F32, tag=f"w1f{k}")
                nc.scalar.dma_start(w1f[:], w1_d[k][:])
                w1 = singles.tile([X_DIM, H2], F16, tag=f"w1h{k}")
                nc.vector.tensor_copy(w1[:], w1f[:])
                w1h.append(w1)

                w2f = singles.tile([128, 4, D], F32, tag=f"w2f{k}")
                for c in range(4):
                    nc.scalar.dma_start(w2f[:, c, :], w2_d[k][c * 128:(c + 1) * 128, :])
                w2 = singles.tile([128, 4, D], F16, tag=f"w2h{k}")
                nc.vector.tensor_copy(w2[:], w2f[:])
                w2h.append(w2)

                bt = []
                for c in range(4):
                    b = singles.tile([128, 1], F32, tag=f"b1_{k}_{c}")
                    nc.scalar.dma_start(b[:], b1_d[k][c * 128:(c + 1) * 128].rearrange("(p one) -> p one", one=1))
                    bt.append(b)
                b1t.append(bt)

                b2 = singles.tile([D, 1], F32, tag=f"b2_{k}")
                nc.scalar.dma_start(b2[:], b2_d[k][:].rearrange("(p one) -> p one", one=1))
                b2t.append(b2)
                nb2 = singles.tile([D, 1], F32, tag=f"nb2_{k}")
                nc.vector.tensor_scalar(out=nb2[:], in0=b2[:], scalar1=-1.0,
                                        scalar2=None, op0=OP.mult)
                nb2t.append(nb2)

            # --- per-d partial accumulators, one column per group ---
            acc = {}
            for nm in ("P1", "P2", "C", "B", "S", "T"):
                acc_t = singles.tile([D, NG], F32, tag=f"acc_{nm}", name=f"acc_{nm}")
                acc[nm] = acc_t

            for g in range(NG):
                if g == 0:
                    xT, yT = xT0, yT0
                else:
                    xT = tposed.tile([X_DIM, RG], F16, tag="xT")
                    yT = tposed.tile([D, RG], F16, tag="yT")
                    cast_and_transpose(g, xT, yT, nsub=1)

                # --- layer 1 + leaky ---
                hT = []
                for k in range(2):
                    hk = []
                    for c in range(4):
                        hp = hpsum.tile([128, RG], F32, tag="hps")
                        for s in range(RG // 512):
                            nc.tensor.matmul(hp[:, s * 512:(s + 1) * 512],
                                             w1h[k][:, c * 128:(c + 1) * 128],
                                             xT[:, s * 512:(s + 1) * 512],
                                             start=True, stop=True)
                        ht = hidden.tile([128, RG], F16, tag=f"hT{k}{c}")
                        if _dve_leaky(g, k, c):
                            nc.vector._custom_dve(
                                _get_leaky_op(), out=ht[:], in0=hp[:],
                                s0=b1t[k][c][:], imm2=NEG_SLOPE)
                        else:
                            nc.scalar.activation(ht[:], hp[:], AF.Prelu,
                                                 bias=b1t[k][c][:], scale=1.0,
                                                 alpha=NEG_SLOPE)
                        hk.append(ht)
                    hT.append(hk)

                # --- layer 2 (accumulate over 4 chunks) ---
                mups = l2psum.tile([D, RG], F32, tag="mups")
                zps = l2psum.tile([D, RG], F32, tag="zps")
                for s in range(RG // 512):
                    for c in range(4):
                        nc.tensor.matmul(mups[:, s * 512:(s + 1) * 512],
                                         w2h[0][:, c, :],
                                         hT[0][c][:, s * 512:(s + 1) * 512],
                                         start=(c == 0), stop=(c == 3))
                for s in range(RG // 512):
                    for c in range(4):
                        nc.tensor.matmul(zps[:, s * 512:(s + 1) * 512],
                                         w2h[1][:, c, :],
                                         hT[1][c][:, s * 512:(s + 1) * 512],
                                         start=(c == 0), stop=(c == 3))

                # mu = psum + b2m (fp32); u = -tanh(psum + b2v); iv = exp(u)
                mu = l2pool.tile([D, RG], F32, tag="mu")
                nc.scalar.activation(mu[:], mups[:], AF.Identity, bias=b2t[0][:])
                u = l2pool.tile([D, RG], F32, tag="u")
                nc.scalar.activation(u[:], zps[:], AF.Tanh, bias=nb2t[1][:], scale=-1.0)
                iv = l2pool.tile([D, RG], F32, tag="iv")
                nc.scalar.activation(iv[:], u[:], AF.Exp,
                                     accum_out=acc["B"][:, g:g + 1])

                # --- product stage (fp32 internal, fp32 accumulators) ---
                q = scratch.tile([D, RG], F32, tag="q")
                nc.vector.scalar_tensor_tensor(
                    out=q[:], in0=iv[:], scalar=1.0, in1=mu[:],
                    op0=OP.mult, op1=OP.mult,
                    accum_out=acc["C"][:, g:g + 1])
                p2s = scratch.tile([D, RG], F32, tag="p2s")
                _peng(nc).scalar_tensor_tensor(
                    out=p2s[:], in0=q[:], scalar=1.0, in1=yT[:],
                    op0=OP.mult, op1=OP.mult,
                    accum_out=acc["P2"][:, g:g + 1])
                y2 = scratch.tile([D, RG], F16, tag="y2")
                nc.vector.scalar_tensor_tensor(
                    out=y2[:], in0=yT[:], scalar=1.0, in1=yT[:],
                    op0=OP.mult, op1=OP.mult)
                # T must sum the SAME fp16-rounded y2 tile P1 consumes:
                # fp16(y^2) rounding is biased, and in the combine the bias
                # only cancels against P1's if T carries it too.
                t2s = scratch.tile([D, RG], F16, tag="t2s")
                nc.vector.tensor_scalar(
                    out=t2s[:], in0=y2[:], scalar1=1.0, scalar2=None,
                    op0=OP.mult, op1=OP.add,
                    accum_out=acc["T"][:, g:g + 1])
                p1s = scratch.tile([D, RG], F32, tag="p1s")
                _peng(nc).scalar_tensor_tensor(
                    out=p1s[:], in0=iv[:], scalar=1.0, in1=y2[:],
                    op0=OP.mult, op1=OP.mult,
                    accum_out=acc["P1"][:, g:g + 1])
                ss = scratch.tile([D, RG], F16, tag="ss")
                # out = y*1; accum reduces out with op1 (add) along free dim
                nc.vector.tensor_scalar(
                    out=ss[:], in0=yT[:], scalar1=1.0, scalar2=None,
                    op0=OP.mult, op1=OP.add,
                    accum_out=acc["S"][:, g:g + 1])

            for i, nm in enumerate(("P1", "P2", "C", "B", "S", "T")):
                nc.sync.dma_start(out_d[i], acc[nm][:])

    nc.compile()
    return nc


def _get_compiled():
    global _compiled
    if _compiled is None:
        _compiled = _build()
    return _compiled


def kernel(x_samples, y_samples, W1m, b1m, W2m, b2m, W1v, b1v, W2v, b2v):
    from concourse.bass_utils import run_bass_kernel_spmd

    nc = _get_compiled()

    xs = np.ascontiguousarray(x_samples, dtype=np.float32)
    ys = np.ascontiguousarray(y_samples, dtype=np.float32)
    in_maps = []
    for i in range(N_CORES):
        sl = slice(i * M, (i + 1) * M)
        in_maps.append({
            "x": xs[sl], "y": ys[sl],
            "W1m": np.asarray(W1m, np.float32), "b1m": np.asarray(b1m, np.float32),
            "W2m": np.asarray(W2m, np.float32), "b2m": np.asarray(b2m, np.float32),
            "W1v": np.asarray(W1v, np.float32), "b1v": np.asarray(b1v, np.float32),
            "W2v": np.asarray(W2v, np.float32), "b2v": np.asarray(b2v, np.float32),
        })

    res = run_bass_kernel_spmd(nc, in_maps, list(range(N_CORES)))
    return combine([r["out"] for r in res.results])


def combine(outs):
    """Host-side gather: sum per-core [6, 128, NG] partials and finish the loss."""
    tot = np.sum([o.astype(np.float64) for o in outs], axis=(0, 3))
    P1, P2, C, B, S, T = tot
    ym = S / N
    y2m = T / N
    total = P1.sum() - 2.0 * P2.sum() - (y2m * B).sum() + 2.0 * (ym * C).sum()
    return np.float32(-0.5 * total / N)



# revision 3
# speedup vs baseline: 1.2192x; 1.2192x over previous
"""Trainium2 Bass kernel for nn_CLUB_816043786555 (CLUB loss).

Full-input contract: kernel(**inputs) takes the complete arrays, shards the
batch dim across 8 NeuronCores, runs a Bass/Tile kernel per core, and
combines tiny per-core partial sums on the host.

Math: with mu = leaky(x@W1m+b1m)@W2m+b2m, logvar = tanh(leaky(x@W1v+b1v)@W2v+b2v),
iv = exp(-logvar), ym_d = mean_i y, y2m_d = mean_i y^2:

  loss = -0.5/N * sum_{i,d} iv*(y^2 - 2*mu*y - y2m + 2*mu*ym)
       = -0.5/N * sum_{i,d} iv*((y^2 - y2m) - 2*mu*(y - ym))
       = -0.5/N * (P1 - 2*P2)

with yc = y - ym and y2c = y^2 - y2m centered ON THE HOST (exact, fp64) and
uploaded as fp16 alongside fp16 x — all pre-transposed to [feature, row]
blocked layout so the device does zero transposes and zero casts.  Per-core
partials P1 = sum iv*y2c and P2 = sum iv*mu*yc are accumulated per group via
engine accum_out in fp32; the host combine is O(NG) work.

The device pipeline per 1024-row group:
  PE:   L1 matmuls (fp16, [128,512] psum half-tiles, 4-deep rotation),
        L2 matmuls (accumulate 4 chunks into [128,1024] psum)
  ACT:  tanh, exp + a share of the leaky evacuations (Prelu w/ bias)
  DVE:  the other leaky evacuations (custom fused bias+leaky uop)
  Pool: mu = psum + b2m, and the three product passes (STT w/ accum_out)
"""

import numpy as np

N_CORES = 8
N = 131072
D = 128
X_DIM = 128
H2 = 512
M = N // N_CORES          # rows per core = 16384
RG = 1024                 # rows per group
NG = M // RG              # groups per core = 16
NEG_SLOPE = 0.2

# Which of the 16 leaky evacuations per group run on ACT (True) vs DVE.
# 16 halves indexed (k, c, s) -> k*8 + c*2 + s.
LEAKY_ON_ACT = tuple((i % 16) in (0, 2, 4, 6, 8, 10, 12) for i in range(16))

# Engine for mu evac + product passes: "pool" or "dve"
MU_ENGINE = "pool"
PROD_ENGINE = "pool"

# Input prefetch depth (tile pool bufs for xt/yct/y2ct)
PREFETCH = 3

_leaky_op = None


def _get_leaky_op():
    """Custom DVE uop: out = max((in0 + s0) * imm2, in0 + s0) — fused
    bias-add + leaky-relu in one 1x pass straight from PSUM."""
    global _leaky_op
    if _leaky_op is not None:
        return _leaky_op
    import concourse.dve_ops as DO
    from concourse.dve_spec import C0, C2, Spec, Src0, maxx

    op = DO.DveOp(
        "LEAKY_BIAS_ANT",
        Spec(
            body=maxx((Src0 + C0) * C2, Src0 + C0),
            reference=lambda in0, in1, s0, s1, imm2: np.maximum(
                (in0.astype(np.float32) + s0) * imm2,
                in0.astype(np.float32) + s0),
        ),
        subdim=False,
        uops_sha={"v3": "28ce115f5da0f06f", "v4": ""},
    )
    DO.OPS.append(op)
    DO.CUSTOM_DVE_SPECS[op.name] = op.spec
    DO._SUB_OPCODE_FOR_NAME[op.name] = DO._CUSTOM_DVE_ROW_BASE + len(DO.OPS) - 1
    assert DO._SUB_OPCODE_FOR_NAME[op.name] < 0x20
    _leaky_op = op
    return op


_compiled = None


def _build():
    import concourse.bacc as bacc
    import concourse.tile as tile
    import concourse.mybir as mybir

    F32 = mybir.dt.float32
    F16 = mybir.dt.float16
    AF = mybir.ActivationFunctionType
    OP = mybir.AluOpType

    nc = bacc.Bacc("TRN2", target_bir_lowering=False, debug=False,
                   num_devices=N_CORES)

    xt_d = nc.dram_tensor("xt", [NG, X_DIM, RG], F16, kind="ExternalInput")
    yct_d = nc.dram_tensor("yct", [NG, D, RG], F16, kind="ExternalInput")
    y2ct_d = nc.dram_tensor("y2ct", [NG, D, RG], F16, kind="ExternalInput")
    w1_d = [nc.dram_tensor("w1m", [X_DIM, H2], F16, kind="ExternalInput"),
            nc.dram_tensor("w1v", [X_DIM, H2], F16, kind="ExternalInput")]
    # pre-chunked on host: [4, 128, 128] with [c, h, d] = W2[c*128+h, d]
    w2_d = [nc.dram_tensor("w2m", [4, 128, D], F16, kind="ExternalInput"),
            nc.dram_tensor("w2v", [4, 128, D], F16, kind="ExternalInput")]
    # [128, 4] f32, column c = b1[c*128:(c+1)*128]
    b1_d = [nc.dram_tensor("b1m", [128, 4], F32, kind="ExternalInput"),
            nc.dram_tensor("b1v", [128, 4], F32, kind="ExternalInput")]
    # [128, 2] f32: col0 = b2m, col1 = -b2v
    b2_d = nc.dram_tensor("b2", [128, 2], F32, kind="ExternalInput")
    out_d = nc.dram_tensor("out", [2, D, NG], F32, kind="ExternalOutput")

    with tile.TileContext(nc) as tc:
        with (
            tc.tile_pool(name="singles", bufs=1) as singles,
            tc.tile_pool(name="inp", bufs=PREFETCH) as inp,
            tc.tile_pool(name="hidden", bufs=2) as hidden,
            tc.tile_pool(name="l2", bufs=2) as l2pool,
            tc.tile_pool(name="scratch", bufs=2) as scratch,
            tc.tile_pool(name="hpsum", bufs=4, space="PSUM") as hpsum,
            tc.tile_pool(name="l2psum", bufs=1, space="PSUM") as l2psum,
        ):
            # ---- group-0 inputs first so they beat the weight loads ----
            def load_inputs(g):
                xt = inp.tile([X_DIM, RG], F16, tag="xt")
                yct = inp.tile([D, RG], F16, tag="yct")
                y2ct = inp.tile([D, RG], F16, tag="y2ct")
                nc.sync.dma_start(xt[:], xt_d[g])
                nc.sync.dma_start(yct[:], yct_d[g])
                nc.sync.dma_start(y2ct[:], y2ct_d[g])
                return xt, yct, y2ct

            tiles = {}
            for g in range(min(PREFETCH - 1, NG)):
                tiles[g] = load_inputs(g)

            # ---- weights / biases (already fp16/pre-chunked on host) ----
            w1t, w2t, b1t = [], [], []
            for k in range(2):
                w1 = singles.tile([X_DIM, H2], F16, tag=f"w1_{k}")
                nc.scalar.dma_start(w1[:], w1_d[k][:])
                w1t.append(w1)
                w2 = singles.tile([128, 4, D], F16, tag=f"w2_{k}")
                nc.scalar.dma_start(w2[:], w2_d[k][:].rearrange("c h d -> h c d"))
                w2t.append(w2)
                b1 = singles.tile([128, 4], F32, tag=f"b1_{k}")
                nc.scalar.dma_start(b1[:], b1_d[k][:])
                b1t.append(b1)
            b2 = singles.tile([128, 2], F32, tag="b2")
            nc.scalar.dma_start(b2[:], b2_d[:])

            accP1 = singles.tile([D, NG], F32, tag="accP1")
            accP2 = singles.tile([D, NG], F32, tag="accP2")

            _prod = nc.gpsimd if PROD_ENGINE == "pool" else nc.vector
            _mu_eng = nc.gpsimd if MU_ENGINE == "pool" else nc.vector

            for g in range(NG):
                xt, yct, y2ct = tiles.pop(g)
                if g + PREFETCH - 1 < NG:
                    tiles[g + PREFETCH - 1] = load_inputs(g + PREFETCH - 1)

                # hT[k][c] : [128, RG] f16 leaky(L1) output
                hT = [[hidden.tile([128, RG], F16, tag=f"hT{k}{c}",
                                   name=f"hT{k}{c}")
                       for c in range(4)] for k in range(2)]

                def l1(k):
                    # halves ordered s-major so L2's s=0 deps complete first
                    for s in range(RG // 512):
                        for c in range(4):
                            hp = hpsum.tile([128, 512], F32, tag="hps")
                            nc.tensor.matmul(hp[:],
                                             w1t[k][:, c * 128:(c + 1) * 128],
                                             xt[:, s * 512:(s + 1) * 512],
                                             start=True, stop=True)
                            dst = hT[k][c][:, s * 512:(s + 1) * 512]
                            if LEAKY_ON_ACT[k * 8 + c * 2 + s]:
                                nc.scalar.activation(
                                    dst, hp[:], AF.Prelu,
                                    bias=b1t[k][:, c:c + 1], scale=1.0,
                                    alpha=NEG_SLOPE)
                            else:
                                nc.vector._custom_dve(
                                    _get_leaky_op(), out=dst, in0=hp[:],
                                    s0=b1t[k][:, c:c + 1], imm2=NEG_SLOPE)

                def l2(k, ps):
                    for s in range(RG // 512):
                        for c in range(4):
                            nc.tensor.matmul(ps[:, s * 512:(s + 1) * 512],
                                             w2t[k][:, c, :],
                                             hT[k][c][:, s * 512:(s + 1) * 512],
                                             start=(c == 0), stop=(c == 3))

                mups = l2psum.tile([D, RG], F32, tag="mups")
                zps = l2psum.tile([D, RG], F32, tag="zps")
                l1(0)
                l2(0, mups)
                l1(1)
                l2(1, zps)

                # mu16 = mups + b2m  (Pool)
                mu16 = l2pool.tile([D, RG], F16, tag="mu16")
                _mu_eng.tensor_scalar(out=mu16[:], in0=mups[:],
                                      scalar1=b2[:, 0:1], scalar2=None,
                                      op0=OP.add)
                # u16 = tanh(-zps - b2v) = -tanh(zps + b2v); iv16 = exp(u16)
                u16 = l2pool.tile([D, RG], F16, tag="u16")
                nc.scalar.activation(u16[:], zps[:], AF.Tanh,
                                     bias=b2[:, 1:2], scale=-1.0)
                iv16 = l2pool.tile([D, RG], F16, tag="iv16")
                nc.scalar.activation(iv16[:], u16[:], AF.Exp)

                # products (Pool STT, fp16 in/out, fp32 accum columns)
                q16 = scratch.tile([D, RG], F16, tag="q16")
                _prod.scalar_tensor_tensor(
                    out=q16[:], in0=iv16[:], scalar=1.0, in1=mu16[:],
                    op0=OP.mult, op1=OP.mult)
                p1o = scratch.tile([D, RG], F16, tag="p1o")
                _prod.scalar_tensor_tensor(
                    out=p1o[:], in0=iv16[:], scalar=1.0, in1=y2ct[:],
                    op0=OP.mult, op1=OP.mult,
                    accum_out=accP1[:, g:g + 1])
                p2o = scratch.tile([D, RG], F16, tag="p2o")
                _prod.scalar_tensor_tensor(
                    out=p2o[:], in0=q16[:], scalar=1.0, in1=yct[:],
                    op0=OP.mult, op1=OP.mult,
                    accum_out=accP2[:, g:g + 1])

            nc.sync.dma_start(out_d[0], accP1[:])
            nc.sync.dma_start(out_d[1], accP2[:])

    nc.compile()
    return nc


def _get_compiled():
    global _compiled
    if _compiled is None:
        _compiled = _build()
    return _compiled


def _prep_host(x_samples, y_samples, W1m, b1m, W2m, b2m, W1v, b1v, W2v, b2v):
    """Host-side preprocessing: center y exactly (fp64), cast to fp16,
    transpose to [feature, row] and block into [NG, 128, RG] per core."""
    x = np.ascontiguousarray(x_samples, dtype=np.float32)
    y = np.asarray(y_samples, dtype=np.float32)
    y64 = y.astype(np.float64)
    ym = y64.mean(axis=0)
    y2m = (y64 * y64).mean(axis=0)
    yc = (y64 - ym).astype(np.float32)
    y2c = (y64 * y64 - y2m).astype(np.float32)

    def block(a):  # [M, 128] f32 -> [NG, 128, RG] f16 (transposed per group)
        return np.ascontiguousarray(
            a.reshape(NG, RG, 128).transpose(0, 2, 1)).astype(np.float16)

    w1m16 = np.asarray(W1m, np.float32).astype(np.float16)
    w1v16 = np.asarray(W1v, np.float32).astype(np.float16)

    def chunk_w2(W2):  # [512, 128] -> [4, 128, 128] f16
        return np.ascontiguousarray(
            np.asarray(W2, np.float32).reshape(4, 128, D)).astype(np.float16)

    def chunk_b1(b1):  # [512] -> [128, 4] f32, col c = b1[c*128:(c+1)*128]
        return np.ascontiguousarray(
            np.asarray(b1, np.float32).reshape(4, 128).T)

    b2c = np.ascontiguousarray(np.stack(
        [np.asarray(b2m, np.float32), -np.asarray(b2v, np.float32)],
        axis=1))

    shared = {
        "w1m": w1m16, "w1v": w1v16,
        "w2m": chunk_w2(W2m), "w2v": chunk_w2(W2v),
        "b1m": chunk_b1(b1m), "b1v": chunk_b1(b1v),
        "b2": b2c,
    }
    in_maps = []
    for i in range(N_CORES):
        sl = slice(i * M, (i + 1) * M)
        m = {"xt": block(x[sl]), "yct": block(yc[sl]),
             "y2ct": block(y2c[sl])}
        m.update(shared)
        in_maps.append(m)
    return in_maps


def kernel(x_samples, y_samples, W1m, b1m, W2m, b2m, W1v, b1v, W2v, b2v):
    from concourse.bass_utils import run_bass_kernel_spmd

    nc = _get_compiled()
    in_maps = _prep_host(x_samples, y_samples, W1m, b1m, W2m, b2m,
                         W1v, b1v, W2v, b2v)
    res = run_bass_kernel_spmd(nc, in_maps, list(range(N_CORES)))
    return combine([r["out"] for r in res.results])


def combine(outs):
    """Host-side gather: sum per-core [2, 128, NG] partials, finish the loss."""
    tot = np.sum([o.astype(np.float64) for o in outs], axis=(0, 2, 3))
    P1, P2 = tot
    return np.float32(-0.5 * (P1 - 2.0 * P2) / N)


# revision 4
# speedup vs baseline: 1.5417x; 1.2645x over previous
"""Trainium2 Bass kernel for nn_CLUB_816043786555 (CLUB loss).

Full-input contract: kernel(**inputs) takes the complete arrays, shards the
batch dim across 8 NeuronCores, runs a Bass/Tile kernel per core, and
combines tiny per-core partial sums on the host.

Math: with mu = leaky(x@W1m+b1m)@W2m+b2m, logvar = tanh(leaky(x@W1v+b1v)@W2v+b2v),
iv = exp(-logvar), ym_d = mean_i y, y2m_d = mean_i y^2:

  loss = -0.5/N * sum_{i,d} iv*((y^2 - y2m) - 2*mu*(y - ym))
       = -0.5/N * (P1 - 2*P2)

with yc = y - ym and y2c = y^2 - y2m centered ON THE HOST (exact, fp64) and
uploaded as fp16 alongside fp16 x — all pre-transposed to [feature, row]
blocked layout so the device does zero transposes and zero casts.  Per-core
partials P1 = sum iv*y2c and P2 = sum iv*mu*yc are accumulated per group via
engine accum_out in fp32; the host combine is O(NG) work.

Per 1024-row group the device runs a 4-engine balanced pipeline (~6.5-6.8us
each): PE does L1/L2 fp16 matmuls ([128,512] psum tiles, deep rotation);
leaky evacuations split 7/9 between ACT (Prelu w/ bias) and DVE (custom
fused bias+leaky uop); ACT also does tanh halves + exp; Pool does the mu
bias-add halves and the two accumulating product passes; DVE does the
q = iv*mu product as a 2x-mode fp16 tensor_tensor.
"""

import numpy as np

N_CORES = 8
N = 131072
D = 128
X_DIM = 128
H2 = 512
M = N // N_CORES          # rows per core = 16384
RG = 1024                 # rows per group
NG = M // RG              # groups per core = 16
NEG_SLOPE = 0.2

# Which of the 16 leaky evacuations per group run on ACT (True) vs DVE.
# 16 halves indexed (k, s, c) in PE emission order.
LEAKY_ON_ACT = tuple(i % 16 in (0, 2, 4, 7, 9, 11, 13) for i in range(16))

# Input prefetch depth (tile pool bufs for xt/yct/y2ct)
PREFETCH = 3
HPSUM_BUFS = 4
L2PSUM_BUFS = 2

_leaky_op = None


def _get_leaky_op():
    """Custom DVE uop: out = max((in0 + s0) * imm2, in0 + s0) — fused
    bias-add + leaky-relu in one 1x pass straight from PSUM."""
    global _leaky_op
    if _leaky_op is not None:
        return _leaky_op
    import concourse.dve_ops as DO
    from concourse.dve_spec import C0, C2, Spec, Src0, maxx

    op = DO.DveOp(
        "LEAKY_BIAS_ANT",
        Spec(
            body=maxx((Src0 + C0) * C2, Src0 + C0),
            reference=lambda in0, in1, s0, s1, imm2: np.maximum(
                (in0.astype(np.float32) + s0) * imm2,
                in0.astype(np.float32) + s0),
        ),
        subdim=False,
        uops_sha={"v3": "28ce115f5da0f06f", "v4": ""},
    )
    DO.OPS.append(op)
    DO.CUSTOM_DVE_SPECS[op.name] = op.spec
    DO._SUB_OPCODE_FOR_NAME[op.name] = DO._CUSTOM_DVE_ROW_BASE + len(DO.OPS) - 1
    assert DO._SUB_OPCODE_FOR_NAME[op.name] < 0x20
    _leaky_op = op
    return op


_compiled = None


def _build():
    import concourse.bacc as bacc
    import concourse.tile as tile
    import concourse.mybir as mybir

    F32 = mybir.dt.float32
    F16 = mybir.dt.float16
    AF = mybir.ActivationFunctionType
    OP = mybir.AluOpType

    nc = bacc.Bacc("TRN2", target_bir_lowering=False, debug=False,
                   num_devices=N_CORES)

    xt_d = nc.dram_tensor("xt", [NG, X_DIM, RG], F16, kind="ExternalInput")
    yct_d = nc.dram_tensor("yct", [NG, D, RG], F16, kind="ExternalInput")
    y2ct_d = nc.dram_tensor("y2ct", [NG, D, RG], F16, kind="ExternalInput")
    w1_d = [nc.dram_tensor("w1m", [X_DIM, H2], F16, kind="ExternalInput"),
            nc.dram_tensor("w1v", [X_DIM, H2], F16, kind="ExternalInput")]
    # pre-chunked on host: [4, 128, 128] with [c, h, d] = W2[c*128+h, d]
    w2_d = [nc.dram_tensor("w2m", [4, 128, D], F16, kind="ExternalInput"),
            nc.dram_tensor("w2v", [4, 128, D], F16, kind="ExternalInput")]
    # [128, 4] f32, column c = b1[c*128:(c+1)*128]
    b1_d = [nc.dram_tensor("b1m", [128, 4], F32, kind="ExternalInput"),
            nc.dram_tensor("b1v", [128, 4], F32, kind="ExternalInput")]
    # [128, 2] f32: col0 = b2m, col1 = -b2v
    b2_d = nc.dram_tensor("b2", [128, 2], F32, kind="ExternalInput")
    out_d = nc.dram_tensor("out", [2, D, NG], F32, kind="ExternalOutput")

    with tile.TileContext(nc) as tc:
        with (
            tc.tile_pool(name="singles", bufs=1) as singles,
            tc.tile_pool(name="inp", bufs=PREFETCH) as inp,
            tc.tile_pool(name="hidden", bufs=2) as hidden,
            tc.tile_pool(name="l2", bufs=2) as l2pool,
            tc.tile_pool(name="scratch", bufs=2) as scratch,
            tc.tile_pool(name="hpsum", bufs=HPSUM_BUFS, space="PSUM") as hpsum,
            tc.tile_pool(name="l2psum", bufs=L2PSUM_BUFS, space="PSUM") as l2psum,
        ):
            # ---- group-0 x first so the first matmul starts ASAP ----
            def load_x(g):
                xt = inp.tile([X_DIM, RG], F16, tag="xt", name="xt")
                nc.sync.dma_start(xt[:], xt_d[g])
                return xt

            def load_y(g):
                yct = inp.tile([D, RG], F16, tag="yct", name="yct")
                y2ct = inp.tile([D, RG], F16, tag="y2ct", name="y2ct")
                nc.sync.dma_start(yct[:], yct_d[g])
                nc.sync.dma_start(y2ct[:], y2ct_d[g])
                return yct, y2ct

            xtiles = {0: load_x(0)}

            # ---- weights / biases via Pool SWDGE (keeps HWDGE free) ----
            w1t, w2t, b1t = [], [], []
            for k in range(2):
                w1 = singles.tile([X_DIM, H2], F16, tag=f"w1_{k}")
                nc.gpsimd.dma_start(w1[:], w1_d[k][:])
                w1t.append(w1)
                b1 = singles.tile([128, 4], F32, tag=f"b1_{k}")
                nc.gpsimd.dma_start(b1[:], b1_d[k][:])
                b1t.append(b1)
                w2 = singles.tile([128, 4, D], F16, tag=f"w2_{k}")
                nc.gpsimd.dma_start(w2[:], w2_d[k][:].rearrange("c h d -> h c d"))
                w2t.append(w2)
            b2 = singles.tile([128, 2], F32, tag="b2")
            nc.gpsimd.dma_start(b2[:], b2_d[:])

            ytiles = {0: load_y(0)}
            for g in range(1, min(PREFETCH - 1, NG)):
                xtiles[g] = load_x(g)
                ytiles[g] = load_y(g)

            accP1 = singles.tile([D, NG], F32, tag="accP1")
            accP2 = singles.tile([D, NG], F32, tag="accP2")

            for g in range(NG):
                xt = xtiles.pop(g)
                yct, y2ct = ytiles.pop(g)
                if g + PREFETCH - 1 < NG:
                    gp = g + PREFETCH - 1
                    xtiles[gp] = load_x(gp)
                    ytiles[gp] = load_y(gp)

                # hT[k][c] : [128, RG] f16 leaky(L1) output
                hT = [[hidden.tile([128, RG], F16, tag=f"hT{k}{c}",
                                   name=f"hT{k}{c}")
                       for c in range(4)] for k in range(2)]

                nleaky = 0

                def l1(k):
                    nonlocal nleaky
                    # halves ordered s-major so L2's s=0 deps complete first
                    for s in range(RG // 512):
                        for c in range(4):
                            hp = hpsum.tile([128, 512], F32, tag="hps")
                            nc.tensor.matmul(hp[:],
                                             w1t[k][:, c * 128:(c + 1) * 128],
                                             xt[:, s * 512:(s + 1) * 512],
                                             start=True, stop=True)
                            dst = hT[k][c][:, s * 512:(s + 1) * 512]
                            if LEAKY_ON_ACT[nleaky]:
                                nc.scalar.activation(
                                    dst, hp[:], AF.Prelu,
                                    bias=b1t[k][:, c:c + 1], scale=1.0,
                                    alpha=NEG_SLOPE)
                            else:
                                nc.vector._custom_dve(
                                    _get_leaky_op(), out=dst, in0=hp[:],
                                    s0=b1t[k][:, c:c + 1], imm2=NEG_SLOPE)
                            nleaky += 1

                def l2(k, s, ps):
                    for c in range(4):
                        nc.tensor.matmul(ps[:],
                                         w2t[k][:, c, :],
                                         hT[k][c][:, s * 512:(s + 1) * 512],
                                         start=(c == 0), stop=(c == 3))

                # --- m-MLP: L1, L2 halves, mu evac (Pool, frees psum early)
                mu16 = l2pool.tile([D, RG], F16, tag="mu16")
                l1(0)
                for s in range(RG // 512):
                    mups = l2psum.tile([D, 512], F32, tag="mups")
                    l2(0, s, mups)
                    nc.gpsimd.tensor_scalar(
                        out=mu16[:, s * 512:(s + 1) * 512], in0=mups[:],
                        scalar1=b2[:, 0:1], scalar2=None, op0=OP.add)

                # --- v-MLP: L1, L2 halves, tanh evac (ACT)
                u16 = l2pool.tile([D, RG], F16, tag="u16")
                l1(1)
                for s in range(RG // 512):
                    zps = l2psum.tile([D, 512], F32, tag="zps")
                    l2(1, s, zps)
                    # u = tanh(-z - b2v) = -tanh(z + b2v)
                    nc.scalar.activation(u16[:, s * 512:(s + 1) * 512],
                                         zps[:], AF.Tanh,
                                         bias=b2[:, 1:2], scale=-1.0)

                iv16 = l2pool.tile([D, RG], F16, tag="iv16")
                nc.scalar.activation(iv16[:], u16[:], AF.Exp)

                # products: q on DVE (fp16 2x mode), p1/p2 on Pool w/ accum
                q16 = scratch.tile([D, RG], F16, tag="q16")
                nc.vector.tensor_tensor(out=q16[:], in0=iv16[:], in1=mu16[:],
                                        op=OP.mult)
                p1o = scratch.tile([D, RG], F16, tag="p1o")
                nc.gpsimd.scalar_tensor_tensor(
                    out=p1o[:], in0=iv16[:], scalar=1.0, in1=y2ct[:],
                    op0=OP.mult, op1=OP.mult,
                    accum_out=accP1[:, g:g + 1])
                p2o = scratch.tile([D, RG], F16, tag="p2o")
                nc.gpsimd.scalar_tensor_tensor(
                    out=p2o[:], in0=q16[:], scalar=1.0, in1=yct[:],
                    op0=OP.mult, op1=OP.mult,
                    accum_out=accP2[:, g:g + 1])

            nc.sync.dma_start(out_d[0], accP1[:])
            nc.sync.dma_start(out_d[1], accP2[:])

    nc.compile()
    return nc


def _get_compiled():
    global _compiled
    if _compiled is None:
        _compiled = _build()
    return _compiled


def _prep_host(x_samples, y_samples, W1m, b1m, W2m, b2m, W1v, b1v, W2v, b2v):
    """Host-side preprocessing: center y exactly (fp64), cast to fp16,
    transpose to [feature, row] and block into [NG, 128, RG] per core."""
    x = np.ascontiguousarray(x_samples, dtype=np.float32)
    y = np.asarray(y_samples, dtype=np.float32)
    y64 = y.astype(np.float64)
    ym = y64.mean(axis=0)
    y2m = (y64 * y64).mean(axis=0)
    yc = (y64 - ym).astype(np.float32)
    y2c = (y64 * y64 - y2m).astype(np.float32)

    def block(a):  # [M, 128] f32 -> [NG, 128, RG] f16 (transposed per group)
        return np.ascontiguousarray(
            a.reshape(NG, RG, 128).transpose(0, 2, 1)).astype(np.float16)

    w1m16 = np.asarray(W1m, np.float32).astype(np.float16)
    w1v16 = np.asarray(W1v, np.float32).astype(np.float16)

    def chunk_w2(W2):  # [512, 128] -> [4, 128, 128] f16
        return np.ascontiguousarray(
            np.asarray(W2, np.float32).reshape(4, 128, D)).astype(np.float16)

    def chunk_b1(b1):  # [512] -> [128, 4] f32, col c = b1[c*128:(c+1)*128]
        return np.ascontiguousarray(
            np.asarray(b1, np.float32).reshape(4, 128).T)

    b2c = np.ascontiguousarray(np.stack(
        [np.asarray(b2m, np.float32), -np.asarray(b2v, np.float32)],
        axis=1))

    shared = {
        "w1m": w1m16, "w1v": w1v16,
        "w2m": chunk_w2(W2m), "w2v": chunk_w2(W2v),
        "b1m": chunk_b1(b1m), "b1v": chunk_b1(b1v),
        "b2": b2c,
    }
    in_maps = []
    for i in range(N_CORES):
        sl = slice(i * M, (i + 1) * M)
        m = {"xt": block(x[sl]), "yct": block(yc[sl]),
             "y2ct": block(y2c[sl])}
        m.update(shared)
        in_maps.append(m)
    return in_maps


def kernel(x_samples, y_samples, W1m, b1m, W2m, b2m, W1v, b1v, W2v, b2v):
    from concourse.bass_utils import run_bass_kernel_spmd

    nc = _get_compiled()
    in_maps = _prep_host(x_samples, y_samples, W1m, b1m, W2m, b2m,
                         W1v, b1v, W2v, b2v)
    res = run_bass_kernel_spmd(nc, in_maps, list(range(N_CORES)))
    return combine([r["out"] for r in res.results])


def combine(outs):
    """Host-side gather: sum per-core [2, 128, NG] partials, finish the loss."""
    tot = np.sum([o.astype(np.float64) for o in outs], axis=(0, 2, 3))
    P1, P2 = tot
    return np.float32(-0.5 * (P1 - 2.0 * P2) / N)
